# revision 3
# baseline (speedup 1.0000x reference)
"""Trainium2 Bass kernel for nn_Attention_55671366091368 (moe_routing).

Sharding: 8 cores = data-parallel over B (2) x tensor-parallel over the 4
branches. Core c handles (b = c//4, branch n = c%4).

Per core:
  qT = rope(Wq_n^T @ a_b^T), kT = rope(Wk^T @ x_b^T)   [feature-major, f32]
  attT[s, t] = kT^T q  (s-blocks of 128, t-chunks of 512, causal blocks only)
  AllReduce(max) of attT over the 4 branch cores (per t-chunk)
  routing: comb = exp(att_max/sqrt(C)) * causal_mask * (att_local == att_max)
  denom[t] = sum_s exp_causal  (ones-matmul)
  yT[c, t] = v^T comb  (bf16), scaled by 1/denom
  ReduceScatter(add) of yT (chunk-major) -> each core owns t-chunk n
  out_chunk = yT_chunk^T @ Wo  (bf16), host reassembles.

Keys layout: s = 0..2047 regular keys (key j visible to query t iff j <= t),
then 4 sink rows (always visible). Sinks are NOT roped; v has no rope.
"""

import os
import numpy as np
import ml_dtypes

import concourse.bass as bass
import concourse.mybir as mybir
import concourse.tile as tile
from concourse import bacc
from concourse.bass_utils import run_bass_kernel_spmd

F32 = mybir.dt.float32
F32R = mybir.dt.float32r
BF16 = mybir.dt.bfloat16
AF = mybir.ActivationFunctionType
OP = mybir.AluOpType

P = 128
B, T, C = 2, 2048, 1024
NB, NS = 4, 4
HALF = C // 2
TC = 512           # t-chunk size
NCH = T // TC      # 4 chunks
CT = C // P        # 8 c-tiles
JT = C // P        # 8 contraction tiles
N_CORES = 8
GROUPS = [[0, 1, 2, 3], [4, 5, 6, 7]]
SQRT_C_INV = 1.0 / 32.0

# att-path matmul dtype: float32r (TF32-like, 4x faster) or float32 (exact).
USE_F32R = os.environ.get("ATT_F32", "0") != "1"


def _att(ap):
    """View an f32 AP as the att-path matmul dtype."""
    return ap.bitcast(F32R) if USE_F32R else ap


def _r8(h):
    """[K*128, N] dram AP -> [128, K, N] partition-major view."""
    return h[:].rearrange("(o p) n -> p o n", p=P)


def build():
    nc = bacc.Bacc(num_devices=N_CORES)

    # ---- per-core external I/O ----
    aT = nc.dram_tensor("aT", [C, T], F32, kind="ExternalInput")
    xT = nc.dram_tensor("xT", [C, T], F32, kind="ExternalInput")
    wq = nc.dram_tensor("wq", [C, C], F32, kind="ExternalInput")
    wk = nc.dram_tensor("wk", [C, C], F32, kind="ExternalInput")
    wv = nc.dram_tensor("wv", [C, C], BF16, kind="ExternalInput")
    wo = nc.dram_tensor("wo", [C, C], BF16, kind="ExternalInput")
    ksink = nc.dram_tensor("ksink", [C, NS], F32, kind="ExternalInput")
    vsink = nc.dram_tensor("vsink", [NS, C], BF16, kind="ExternalInput")
    cosT = nc.dram_tensor("cosT", [HALF, T], F32, kind="ExternalInput")
    sinT = nc.dram_tensor("sinT", [HALF, T], F32, kind="ExternalInput")
    dmask = nc.dram_tensor("dmask", [TC, TC], BF16, kind="ExternalInput")
    out = nc.dram_tensor("out", [TC, C], F32, kind="ExternalOutput")

    with tile.TileContext(nc) as tc:
        with (
            tc.tile_pool(name="pw", bufs=1) as pw,
            tc.tile_pool(name="pv", bufs=1) as pv,
            tc.tile_pool(name="pstream", bufs=1) as pstream,
            tc.tile_pool(name="pcs", bufs=1) as pcs,
            tc.tile_pool(name="ptmp", bufs=3) as ptmp,
            tc.tile_pool(name="pq", bufs=1) as pq,
            tc.tile_pool(name="pkst", bufs=2) as pkst,
            tc.tile_pool(name="psq", bufs=2, space="PSUM") as psq,
            tc.tile_pool(name="psatt", bufs=2, space="PSUM") as psatt,
            tc.tile_pool(name="psden", bufs=1, space="PSUM") as psden,
            tc.tile_pool(name="psy", bufs=2, space="PSUM") as psy,
            tc.tile_pool(name="pdram", bufs=1, space="DRAM") as pdram,
        ):
            # ---------- persistent weights / constants ----------
            ksink_sb = pw.tile([P, CT, NS], F32, name="ksink_sb")
            nc.sync.dma_start(ksink_sb[:], _r8(ksink))
            vsink_sb = pw.tile([NS, C], BF16, name="vsink_sb")
            nc.sync.dma_start(vsink_sb[:], vsink[:])
            dmask_sb = pw.tile([P, 4, TC], BF16, name="dmask_sb")
            nc.sync.dma_start(dmask_sb[:], _r8(dmask))
            ones_sb = pw.tile([P, 1], BF16, name="ones_sb")
            nc.vector.memset(ones_sb[:], 1.0)

            # internal DRAM
            ktd = [
                pdram.tile([C, TC], F32, name=f"ktd{j}", tag=f"ktd{j}")
                for j in range(NCH)
            ]
            arin = [
                pdram.tile([(j + 1) * TC + NS, TC], F32, name=f"arin{j}", tag=f"arin{j}")
                for j in range(NCH)
            ]
            arout = [
                pdram.tile([(j + 1) * TC + NS, TC], F32, name=f"arout{j}", tag=f"arout{j}")
                for j in range(NCH)
            ]
            rs_in = pdram.tile([NCH * C, TC], BF16, name="rs_in")
            rs_out = pdram.tile([C, TC], BF16, name="rs_out")

            # ---------- phase K: kT = rope(Wk^T x^T), stored to DRAM ----------
            if True:
                wk_sb = pw.tile([P, JT, C], F32, name="wk_sb", tag="wqk")
                nc.sync.dma_start(wk_sb[:], _r8(wk))
                for r in range(NCH):
                    xt = pstream.tile([P, JT, TC], F32, name="xat", tag="xat", bufs=2)
                    nc.sync.dma_start(xt[:], _r8(xT)[:, :, r * TC : (r + 1) * TC])
                    for ci in range(4):
                        cos_t = pcs.tile([P, TC], F32, name="cos_t", tag="cos")
                        nc.sync.dma_start(
                            cos_t[:],
                            _r8(cosT)[:, ci, r * TC : (r + 1) * TC],
                        )
                        sin_t = pcs.tile([P, TC], F32, name="sin_t", tag="sin")
                        nc.sync.dma_start(
                            sin_t[:],
                            _r8(sinT)[:, ci, r * TC : (r + 1) * TC],
                        )
                        ps_lo = psq.tile([P, TC], F32, name="ps_lo", tag="psq")
                        for j in range(JT):
                            nc.tensor.matmul(
                                ps_lo[:],
                                wk_sb[:, j, ci * P : (ci + 1) * P],
                                xt[:, j, :],
                                start=(j == 0),
                                stop=(j == JT - 1),
                            )
                        ps_hi = psq.tile([P, TC], F32, name="ps_hi", tag="psq")
                        for j in range(JT):
                            nc.tensor.matmul(
                                ps_hi[:],
                                wk_sb[:, j, (ci + 4) * P : (ci + 5) * P],
                                xt[:, j, :],
                                start=(j == 0),
                                stop=(j == JT - 1),
                            )
                        # rope: lo' = lo*cos - hi*sin ; hi' = lo*sin + hi*cos
                        t1 = ptmp.tile([P, TC], F32, name="t1", tag="rt")
                        nc.vector.tensor_tensor(t1[:], ps_lo[:], cos_t[:], OP.mult)
                        t2 = ptmp.tile([P, TC], F32, name="t2", tag="rt")
                        nc.vector.tensor_tensor(t2[:], ps_lo[:], sin_t[:], OP.mult)
                        t3 = ptmp.tile([P, TC], F32, name="t3", tag="rt")
                        nc.vector.tensor_tensor(t3[:], ps_hi[:], sin_t[:], OP.mult)
                        ko_lo = ptmp.tile([P, TC], F32, name="ko_lo", tag="ko", bufs=2)
                        nc.vector.tensor_tensor(ko_lo[:], t1[:], t3[:], OP.subtract)
                        t4 = ptmp.tile([P, TC], F32, name="t4", tag="rt")
                        nc.vector.tensor_tensor(t4[:], ps_hi[:], cos_t[:], OP.mult)
                        ko_hi = ptmp.tile([P, TC], F32, name="ko_hi", tag="ko", bufs=2)
                        nc.vector.tensor_tensor(ko_hi[:], t2[:], t4[:], OP.add)
                        nc.sync.dma_start(
                            ktd[r][ci * P : (ci + 1) * P, :], ko_lo[:]
                        )
                        nc.sync.dma_start(
                            ktd[r][(ci + 4) * P : (ci + 5) * P, :], ko_hi[:]
                        )

            # ---------- phase V: v = a @ Wv (bf16), resident in SBUF ----------
            v_sb = {}
            pV_ctx = tc.tile_pool(name="pV", bufs=1)
            pV = pV_ctx.__enter__()
            wv_sb = pV.tile([P, JT, C], BF16, name="wv_sb")
            nc.sync.dma_start(wv_sb[:], _r8(wv))
            for r in range(NCH):
                at = pstream.tile([P, JT, TC], F32, name="xat", tag="xat", bufs=2)
                nc.sync.dma_start(at[:], _r8(aT)[:, :, r * TC : (r + 1) * TC])
                abf = pstream.tile([P, JT, TC], BF16, name="abf", tag="abf")
                nc.vector.tensor_copy(abf[:], at[:])
                for sl in range(4):
                    sb = 4 * r + sl
                    vt = pv.tile([P, C], BF16, name=f"v{sb}", tag=f"v{sb}")
                    for ch in range(2):
                        ps_v = psy.tile([P, TC], F32, name="ps_v", tag="psy")
                        for j in range(JT):
                            nc.tensor.matmul(
                                ps_v[:],
                                abf[:, j, sl * P : (sl + 1) * P],
                                wv_sb[:, j, ch * TC : (ch + 1) * TC],
                                start=(j == 0),
                                stop=(j == JT - 1),
                            )
                        nc.vector.tensor_copy(vt[:, ch * TC : (ch + 1) * TC], ps_v[:])
                    v_sb[sb] = vt
            pV_ctx.__exit__(None, None, None)

            # ---------- chunk loop (reverse order: big ARs overlap best) ----------
            pchunk_ctx = tc.tile_pool(name="ppost", bufs=3)
            ppost = pchunk_ctx.__enter__()
            pcomb_ctx = tc.tile_pool(name="pcomb", bufs=1)
            pcomb = pcomb_ctx.__enter__()
            wq_sb = pw.tile([P, JT, C], F32, name="wq_sb", tag="wqk")
            nc.sync.dma_start(wq_sb[:], _r8(wq))

            def emit_q_att(r):
                at = pstream.tile([P, JT, TC], F32, name="xat", tag="xat", bufs=2)
                nc.sync.dma_start(at[:], _r8(aT)[:, :, r * TC : (r + 1) * TC])
                qro = {}
                for ci in range(4):
                    cos_t = pcs.tile([P, TC], F32, name="cos_t", tag="cos")
                    nc.sync.dma_start(
                        cos_t[:], _r8(cosT)[:, ci, r * TC : (r + 1) * TC]
                    )
                    sin_t = pcs.tile([P, TC], F32, name="sin_t", tag="sin")
                    nc.sync.dma_start(
                        sin_t[:], _r8(sinT)[:, ci, r * TC : (r + 1) * TC]
                    )
                    ps_lo = psq.tile([P, TC], F32, name="ps_lo", tag="psq")
                    for j in range(JT):
                        nc.tensor.matmul(
                            ps_lo[:],
                            wq_sb[:, j, ci * P : (ci + 1) * P],
                            at[:, j, :],
                            start=(j == 0),
                            stop=(j == JT - 1),
                        )
                    ps_hi = psq.tile([P, TC], F32, name="ps_hi", tag="psq")
                    for j in range(JT):
                        nc.tensor.matmul(
                            ps_hi[:],
                            wq_sb[:, j, (ci + 4) * P : (ci + 5) * P],
                            at[:, j, :],
                            start=(j == 0),
                            stop=(j == JT - 1),
                        )
                    t1 = ptmp.tile([P, TC], F32, name="t1", tag="rt")
                    nc.vector.tensor_tensor(t1[:], ps_lo[:], cos_t[:], OP.mult)
                    t2 = ptmp.tile([P, TC], F32, name="t2", tag="rt")
                    nc.vector.tensor_tensor(t2[:], ps_lo[:], sin_t[:], OP.mult)
                    t3 = ptmp.tile([P, TC], F32, name="t3", tag="rt")
                    nc.vector.tensor_tensor(t3[:], ps_hi[:], sin_t[:], OP.mult)
                    qlo = pq.tile([P, TC], F32, name=f"q{ci}", tag=f"q{ci}")
                    nc.vector.tensor_tensor(qlo[:], t1[:], t3[:], OP.subtract)
                    t4 = ptmp.tile([P, TC], F32, name="t4", tag="rt")
                    nc.vector.tensor_tensor(t4[:], ps_hi[:], cos_t[:], OP.mult)
                    qhi = pq.tile([P, TC], F32, name=f"q{ci + 4}", tag=f"q{ci + 4}")
                    nc.vector.tensor_tensor(qhi[:], t2[:], t4[:], OP.add)
                    qro[ci] = qlo
                    qro[ci + 4] = qhi
                # att s-blocks
                nsb = 4 * (r + 1)
                for sb in range(nsb):
                    kst = pkst.tile([P, CT, P], F32, name="kst", tag="kst")
                    nc.sync.dma_start(
                        kst[:],
                        ktd[sb // 4][:]
                        .rearrange("(o p) s -> p o s", p=P)[
                            :, :, (sb % 4) * P : (sb % 4 + 1) * P
                        ],
                    )
                    ps_a = psatt.tile([P, TC], F32, name="ps_a", tag="psatt")
                    for ci in range(CT):
                        nc.tensor.matmul(
                            ps_a[:],
                            _att(kst[:, ci, :]),
                            _att(qro[ci][:]),
                            start=(ci == 0),
                            stop=(ci == CT - 1),
                        )
                    asb = ppost.tile([P, TC], F32, name="asb", tag="asb")
                    nc.vector.tensor_copy(asb[:], ps_a[:])
                    nc.sync.dma_start(arin[r][sb * P : (sb + 1) * P, :], asb[:])
                # sinks
                ps_s = psatt.tile([NS, TC], F32, name="ps_s", tag="psatts", bufs=1)
                for ci in range(CT):
                    nc.tensor.matmul(
                        ps_s[:],
                        _att(ksink_sb[:, ci, :]),
                        _att(qro[ci][:]),
                        start=(ci == 0),
                        stop=(ci == CT - 1),
                    )
                asb_s = pcomb.tile([NS, TC], F32, name="asb_s", tag="asb_s")
                nc.vector.tensor_copy(asb_s[:], ps_s[:])
                nc.sync.dma_start(arin[r][nsb * P : nsb * P + NS, :], asb_s[:])
                nc.gpsimd.collective_compute(
                    "AllReduce",
                    OP.max,
                    replica_groups=GROUPS,
                    ins=[arin[r].opt()],
                    outs=[arout[r].opt()],
                )

            def emit_post_y(r):
                nsb = 4 * (r + 1)
                ps_d = psden.tile([1, TC], F32, name="ps_d", tag="psden")
                combs = {}
                for sb in range(nsb):
                    amax = ppost.tile([P, TC], F32, name="amax", tag="amax")
                    nc.sync.dma_start(amax[:], arout[r][sb * P : (sb + 1) * P, :])
                    aloc = ppost.tile([P, TC], F32, name="aloc", tag="aloc")
                    nc.sync.dma_start(aloc[:], arin[r][sb * P : (sb + 1) * P, :])
                    ex = ppost.tile([P, TC], BF16, name="ex", tag="ex")
                    nc.scalar.activation(ex[:], amax[:], AF.Exp, scale=SQRT_C_INV)
                    if sb // 4 == r:  # diagonal superblock: causal mask
                        nc.vector.tensor_tensor(
                            ex[:], ex[:], dmask_sb[:, sb % 4, :], OP.mult
                        )
                    cmb = pcomb.tile([P, TC], BF16, name=f"comb{sb}", tag=f"comb{sb}")
                    nc.vector.tensor_tensor(cmb[:], aloc[:], amax[:], OP.is_equal)
                    nc.vector.tensor_tensor(cmb[:], cmb[:], ex[:], OP.mult)
                    nc.tensor.matmul(
                        ps_d[:], ones_sb[:, :], ex[:], start=(sb == 0), stop=False
                    )
                    combs[sb] = cmb
                # sink rows
                amax_s = pcomb.tile([NS, TC], F32, name="amax_s", tag="amax_s")
                nc.sync.dma_start(amax_s[:], arout[r][nsb * P : nsb * P + NS, :])
                aloc_s = pcomb.tile([NS, TC], F32, name="aloc_s", tag="aloc_s")
                nc.sync.dma_start(aloc_s[:], arin[r][nsb * P : nsb * P + NS, :])
                ex_s = pcomb.tile([NS, TC], BF16, name="ex_s", tag="ex_s")
                nc.scalar.activation(ex_s[:], amax_s[:], AF.Exp, scale=SQRT_C_INV)
                cmb_s = pcomb.tile([NS, TC], BF16, name="comb_s", tag="comb_s")
                nc.vector.tensor_tensor(cmb_s[:], aloc_s[:], amax_s[:], OP.is_equal)
                nc.vector.tensor_tensor(cmb_s[:], cmb_s[:], ex_s[:], OP.mult)
                nc.tensor.matmul(
                    ps_d[:], ones_sb[:NS, :], ex_s[:], start=False, stop=True
                )
                rec = ppost.tile([1, TC], F32, name="rec", tag="rec", bufs=2)
                nc.vector.reciprocal(rec[:], ps_d[:])
                denr = pdram.tile([1, TC], F32, name="denr", tag="denr", bufs=2)
                nc.sync.dma_start(denr[:], rec[:])
                recbc = ppost.tile([P, TC], F32, name="recbc", tag="recbc", bufs=2)
                nc.sync.dma_start(recbc[:], denr[:].to_broadcast((P, TC)))
                # y matmuls
                for ct in range(CT):
                    ps_yt = psy.tile([P, TC], F32, name="ps_yt", tag="psy")
                    for sb in range(nsb):
                        nc.tensor.matmul(
                            ps_yt[:],
                            v_sb[sb][:, ct * P : (ct + 1) * P],
                            combs[sb][:],
                            start=(sb == 0),
                            stop=False,
                        )
                    nc.tensor.matmul(
                        ps_yt[:],
                        vsink_sb[:, ct * P : (ct + 1) * P],
                        cmb_s[:],
                        start=False,
                        stop=True,
                    )
                    ysc = ppost.tile([P, TC], BF16, name="ysc", tag="ysc", bufs=2)
                    nc.vector.tensor_tensor(ysc[:], ps_yt[:], recbc[:], OP.mult)
                    nc.sync.dma_start(
                        rs_in[r * C + ct * P : r * C + (ct + 1) * P, :], ysc[:]
                    )

            order = [3, 2, 1, 0]
            for i, r in enumerate(order):
                emit_q_att(r)
                if i >= 1:
                    emit_post_y(order[i - 1])
            emit_post_y(order[-1])
            pcomb_ctx.__exit__(None, None, None)
            pchunk_ctx.__exit__(None, None, None)

            # ---------- ReduceScatter + o_proj ----------
            nc.gpsimd.collective_compute(
                "ReduceScatter",
                OP.add,
                replica_groups=GROUPS,
                ins=[rs_in.opt()],
                outs=[rs_out.opt()],
            )
            with (
                tc.tile_pool(name="po", bufs=1) as po,
                tc.tile_pool(name="poo", bufs=3) as poo,
            ):
                wo_sb = po.tile([P, CT, C], BF16, name="wo_sb")
                nc.sync.dma_start(wo_sb[:], _r8(wo))
                yrs = po.tile([P, CT, TC], BF16, name="yrs")
                nc.sync.dma_start(yrs[:], rs_out[:].rearrange("(o p) t -> p o t", p=P))
                for tt in range(4):
                    for dh in range(2):
                        ps_o = psy.tile([P, TC], F32, name="ps_o", tag="psy")
                        for ci in range(CT):
                            nc.tensor.matmul(
                                ps_o[:],
                                yrs[:, ci, tt * P : (tt + 1) * P],
                                wo_sb[:, ci, dh * TC : (dh + 1) * TC],
                                start=(ci == 0),
                                stop=(ci == CT - 1),
                            )
                        osb = poo.tile([P, TC], F32, name="osb", tag="osb")
                        nc.vector.tensor_copy(osb[:], ps_o[:])
                        nc.sync.dma_start(
                            out[tt * P : (tt + 1) * P, dh * TC : (dh + 1) * TC],
                            osb[:],
                        )

    nc.compile()
    return nc


_NC_CACHE = None


def _get_nc():
    global _NC_CACHE
    if _NC_CACHE is None:
        _NC_CACHE = build()
    return _NC_CACHE


LAST_EXEC_NS = None
LAST_TRACE = None


def kernel(**inputs):
    global LAST_EXEC_NS, LAST_TRACE
    a = np.ascontiguousarray(np.asarray(inputs["a"], dtype=np.float32))
    x = np.ascontiguousarray(np.asarray(inputs["x"], dtype=np.float32))
    Wq = np.asarray(inputs["Wq"], dtype=np.float32)
    Wk = np.asarray(inputs["Wk"], dtype=np.float32)
    Wv = np.asarray(inputs["Wv"], dtype=np.float32)
    Wo = np.asarray(inputs["Wo"], dtype=np.float32)
    k_sink = np.asarray(inputs["k_sink"], dtype=np.float32)
    v_sink = np.asarray(inputs["v_sink"], dtype=np.float32)

    # rope tables, mirroring the reference's f32 arithmetic
    inv_freq = (
        1.0 / (10000.0 ** (np.arange(HALF, dtype=np.float32) / np.float32(HALF)))
    ).astype(np.float32)
    ang = np.arange(T, dtype=np.float32)[:, None] * inv_freq[None, :]  # (T, HALF)
    cosT = np.ascontiguousarray(np.cos(ang).astype(np.float32).T)  # (HALF, T)
    sinT = np.ascontiguousarray(np.sin(ang).astype(np.float32).T)

    sl = np.arange(TC)[:, None]
    tl = np.arange(TC)[None, :]
    dmask = (sl <= tl).astype(ml_dtypes.bfloat16)

    wo_bf = Wo.astype(ml_dtypes.bfloat16)

    in_maps = []
    for c in range(N_CORES):
        b, n = c // 4, c % 4
        in_maps.append(
            {
                "aT": np.ascontiguousarray(a[b].T),
                "xT": np.ascontiguousarray(x[b].T),
                "wq": np.ascontiguousarray(Wq[:, n * C : (n + 1) * C]),
                "wk": Wk,
                "wv": np.ascontiguousarray(
                    Wv[:, n * C : (n + 1) * C].astype(ml_dtypes.bfloat16)
                ),
                "wo": wo_bf,
                "ksink": np.ascontiguousarray(k_sink[0, 0].T),
                "vsink": np.ascontiguousarray(
                    v_sink[0, n].astype(ml_dtypes.bfloat16)
                ),
                "cosT": cosT,
                "sinT": sinT,
                "dmask": dmask,
            }
        )

    nc = _get_nc()
    trace = os.environ.get("KERNEL_TRACE", "0") == "1"
    kwargs = {}
    if trace:
        import sys, types

        try:
            from trn_agent_boot.trn_boot import _ntff_profile_via_ctypes

            hook = _ntff_profile_via_ctypes("/opt/axon/libaxon_pjrt.so")
            mod = types.ModuleType("antenv.axon_hooks")
            mod.get_axon_ntff_profile_hook = lambda: hook
            sys.modules["antenv.axon_hooks"] = mod
            kwargs["trace"] = True
        except Exception:
            pass

    res = run_bass_kernel_spmd(nc, in_maps, core_ids=list(range(N_CORES)), **kwargs)
    LAST_EXEC_NS = res.exec_time_ns
    LAST_TRACE = (
        res.instructions_and_trace[1] if res.instructions_and_trace else None
    )

    out = np.zeros((B, T, C), dtype=np.float32)
    for c in range(N_CORES):
        b, n = c // 4, c % 4
        out[b, n * TC : (n + 1) * TC, :] = res.results[c]["out"]
    return out
